# revision 26
# baseline (speedup 1.0000x reference)
"""Trainium2 Bass kernel for MultiHeadSelfAttention + BN + residual + LeakyReLU.

Sharding: 8 cores = (batch b, token-half s); each core computes all 8 heads of
attention for its 1152 query tokens of its batch, the full output projection
for those tokens, and BN via a single all-core AllReduce of per-channel
sum/sumsq statistics.

v1 design: the scalar engine's exp stream (166k columns ~ 140us) is the hard
floor; everything else (matmuls, copies, transposes, DMA) is paced to hide
under it. fp8e4 exp output makes the AV weight loads cheap enough that the
tensor engine stays below the scalar engine's rate.
"""

import sys
import types

if "/opt/trn_rl_repo" not in sys.path:
    sys.path.insert(0, "/opt/trn_rl_repo")

import numpy as np
import ml_dtypes

N_CORES = 8
B, C, HH, WW = 4, 256, 48, 48
L = HH * WW            # 2304 tokens per batch
NH, DK = 8, 64
D = NH * DK            # 512
LQ = L // 2            # 1152 query tokens per core
NKT = L // 128         # 18 key tiles
NQT = LQ // 128        # 9 query tiles
STREAM = NKT * LQ      # 20736 logit columns per head
EXP_OP = 1536          # columns per exp ACTIVATE (3 PSUM banks)
N_OPS = (STREAM + EXP_OP - 1) // EXP_OP  # 14 (last op = 768 cols)
NSAMP = B * L          # 9216 samples per channel for BN
BN_EPS = 1e-5
NEG = 0.01

TRACE = False
EXPT_F8 = True         # exp stream dtype: fp8e4 (False -> bf16)

_cache = {}


def _accum_col(qt):
    # packed AV accumulator columns; avoid crossing the 512-col PSUM bank edge
    return qt * 65 if qt < 7 else 512 + (qt - 7) * 65


def _build():
    import concourse.bacc as bacc
    import concourse.mybir as mybir
    import concourse.tile as tile

    dt = mybir.dt
    f32, bf16 = dt.float32, dt.bfloat16
    f8 = dt.float8e4
    e_dt = f8 if EXPT_F8 else bf16
    # exp(x-2) instead of exp(x): scales numerator AND denominator by e^-2
    # (cancels in the softmax) and keeps the max weight ~e^3 well under the
    # fp8e4 max, avoiding overflow->inf->NaN
    EXP_BIAS = -2.0 if EXPT_F8 else 0.0
    Alu = mybir.AluOpType
    Act = mybir.ActivationFunctionType

    nc = bacc.Bacc(trn_type="TRN2", num_devices=N_CORES, debug=False)

    # ---- DRAM I/O ----
    xf_d = nc.dram_tensor("xf", [C, L], bf16, kind="ExternalInput").ap()
    xq_d = nc.dram_tensor("xq", [C, LQ], bf16, kind="ExternalInput").ap()
    wqt_d = nc.dram_tensor("wqt", [C, D], bf16, kind="ExternalInput").ap()
    wkt_d = nc.dram_tensor("wkt", [C, D], bf16, kind="ExternalInput").ap()
    wvt_d = nc.dram_tensor("wvt", [C, NH * 65], bf16, kind="ExternalInput").ap()
    wot_d = nc.dram_tensor("wot", [D, C], bf16, kind="ExternalInput").ap()
    bnw_d = nc.dram_tensor("bnw2", [128, 2], f32, kind="ExternalInput").ap()
    bnb_d = nc.dram_tensor("bnb2", [128, 2], f32, kind="ExternalInput").ap()
    id_d = nc.dram_tensor("ident", [128, 128], bf16, kind="ExternalInput").ap()
    y_d = nc.dram_tensor("y", [C, LQ], f32, kind="ExternalOutput").ap()

    with tile.TileContext(nc) as tc:
        with (
            tc.tile_pool(name="const", bufs=1) as cpool,
            tc.tile_pool(name="big", bufs=1) as big,
            tc.tile_pool(name="pair", bufs=2) as pairp,
            tc.tile_pool(name="scr", bufs=2) as scr,
            tc.tile_pool(name="psA", bufs=2, space="PSUM") as psA,
            tc.tile_pool(name="psB", bufs=1, space="PSUM") as psB,
            tc.tile_pool(name="dram", bufs=1, space="DRAM") as dram,
        ):
            # ---- SBUF constants / inputs ----
            xf_sb = cpool.tile([128, 2, L], bf16)
            xq_sb = cpool.tile([128, 2, LQ], bf16)
            wqt_sb = cpool.tile([128, 2, D], bf16)
            wkt_sb = cpool.tile([128, 2, D], bf16)
            wvt_sb = cpool.tile([128, 2, NH * 65], bf16)
            wot_sb = cpool.tile([128, 4, C], bf16)
            bnw_sb = cpool.tile([128, 2], f32)
            bnb_sb = cpool.tile([128, 2], f32)
            ident_sb = cpool.tile([128, 128], bf16)
            ebias_sb = cpool.tile([128, 1], f32)
            nc.vector.memset(ebias_sb[:], EXP_BIAS)
            # dummy Ln so the auto table-picker chooses the set containing
            # Ln+Exp+Identity+Square -> a single ACT_TABLE_LOAD for the kernel
            ln_warm = cpool.tile([128, 1], f32)
            nc.scalar.activation(ln_warm[:], ebias_sb[:], Act.Ln)

            # weights + query tokens first (first QKV units need them)
            for ct in range(2):
                nc.sync.dma_start(wkt_sb[:, ct, :], wkt_d[ct * 128:(ct + 1) * 128, :])
                nc.sync.dma_start(wqt_sb[:, ct, :], wqt_d[ct * 128:(ct + 1) * 128, :])
                nc.sync.dma_start(wvt_sb[:, ct, :], wvt_d[ct * 128:(ct + 1) * 128, :])
                nc.sync.dma_start(xq_sb[:, ct, :], xq_d[ct * 128:(ct + 1) * 128, :])
                # xf in quarters for earlier availability of early key tiles
                for qchunk in range(4):
                    c0 = qchunk * (L // 4)
                    nc.sync.dma_start(xf_sb[:, ct, c0:c0 + L // 4],
                                      xf_d[ct * 128:(ct + 1) * 128, c0:c0 + L // 4])
            for dtl in range(4):
                nc.sync.dma_start(wot_sb[:, dtl, :], wot_d[dtl * 128:(dtl + 1) * 128, :])
            nc.sync.dma_start(bnw_sb[:], bnw_d[:])
            nc.sync.dma_start(bnb_sb[:], bnb_d[:])
            nc.sync.dma_start(ident_sb[:], id_d[:])

            # ---- big SBUF tensors ----
            k_sb = big.tile([128, 4, L], bf16)        # K: [dpair, pair, kpos]
            q_sb = big.tile([128, 4, LQ], bf16)       # Q
            v_sb = big.tile([128, NKT, NH * 65], bf16)  # V token-major + ones cols
            # 4 exp-stream buffers: pair p writes bufs (p%2)*2+{0,1}; its AV
            # (run one pair-window later) still reads them while the next pair
            # fills the other two
            expt = big.tile([128, 4, STREAM], e_dt)
            attnT = big.tile([128, 4, LQ], bf16)      # transposed attention output
            stats = big.tile([128, 4], f32)
            gstats = big.tile([128, 4], f32)

            # ================= QKV background units =================
            def emit_k_half(dtl, half):
                ps = psA.tile([128, LQ], f32, name="qkvps", tag="stag")
                for ct in range(2):
                    for (q0, qn) in ((0, 512), (512, 512), (1024, 128)):
                        nc.tensor.matmul(
                            ps[:, q0:q0 + qn],
                            wkt_sb[:, ct, dtl * 128:(dtl + 1) * 128],
                            xf_sb[:, ct, half * LQ + q0: half * LQ + q0 + qn],
                            start=(ct == 0), stop=(ct == 1))
                nc.vector.tensor_copy(k_sb[:, dtl, half * LQ:(half + 1) * LQ], ps[:])

            def emit_q_tile(dtl):
                ps = psA.tile([128, LQ], f32, name="qkvps", tag="stag")
                for ct in range(2):
                    for (q0, qn) in ((0, 512), (512, 512), (1024, 128)):
                        nc.tensor.matmul(
                            ps[:, q0:q0 + qn],
                            wqt_sb[:, ct, dtl * 128:(dtl + 1) * 128],
                            xq_sb[:, ct, q0:q0 + qn],
                            start=(ct == 0), stop=(ct == 1))
                nc.vector.tensor_copy(q_sb[:, dtl, :], ps[:])

            def emit_v_ltile(lt):
                ps = psA.tile([128, LQ], f32, name="qkvps", tag="stag")
                for ct in range(2):
                    for (q0, qn) in ((0, 512), (512, 8)):
                        nc.tensor.matmul(
                            ps[:, q0:q0 + qn],
                            xf_sb[:, ct, lt * 128:(lt + 1) * 128],
                            wvt_sb[:, ct, q0:q0 + qn],
                            start=(ct == 0), stop=(ct == 1))
                nc.vector.tensor_copy(v_sb[:, lt, :], ps[:, 0:520])
                # ones columns for the softmax denominator
                nc.vector.memset(v_sb[:, lt, 64::65], 1.0)

            # deadline-ordered background units (global op index g = pr*N_OPS+s;
            # a unit is emitted before the logits of op g). AV of pair pr runs
            # during window pr+1, so V tiles are only consumed from window 1 on.
            units = []
            for lt in range(1, NKT):
                units.append((1 + (2 * lt) // 3, lambda lt=lt: emit_v_ltile(lt)))
            units.append((6, lambda: emit_k_half(0, 1)))
            for pr in range(1, 4):
                units.append((pr * N_OPS - 10, lambda p=pr: emit_k_half(p, 0)))
                units.append((pr * N_OPS - 7, lambda p=pr: emit_q_tile(p)))
                units.append((pr * N_OPS + 6, lambda p=pr: emit_k_half(p, 1)))
            units.sort(key=lambda t: t[0])

            # prologue: what head 0 op 0 needs immediately
            emit_k_half(0, 0)
            emit_q_tile(0)
            emit_v_ltile(0)

            # ================= attention =================
            # AV consumption lags the exp stream by one ACT op, and each
            # head's final ktile + normalize happen inside the NEXT head's
            # window -- the PE never waits on the in-flight ACTIVATE, stays
            # gapless, and the HAM clock gate can hold 2.4GHz.
            apair = {"tile": None}

            def norm_head(ph, acc):
                r0 = (ph % 2) * 64
                if ph % 2 == 0:
                    apair["tile"] = pairp.tile([128, NQT, 128], bf16,
                                               name="apair", tag="apair")
                attn_pair = apair["tile"]
                recA = scr.tile([128, 7], f32, name="recA", tag="recA")
                recB = scr.tile([128, 2], f32, name="recB", tag="recB")
                nc.vector.reciprocal(recA[:], acc[:, 64:64 + 7 * 65:65])
                nc.vector.reciprocal(recB[:], acc[:, 512 + 64:512 + 2 * 65:65])
                accA = acc[:, 0:7 * 65].rearrange("p (q d) -> p q d", d=65)[:, :, 0:64]
                accB = acc[:, 512:512 + 2 * 65].rearrange("p (q d) -> p q d", d=65)[:, :, 0:64]
                nc.vector.tensor_tensor(
                    attn_pair[:, 0:7, r0:r0 + 64], accA,
                    recA[:].unsqueeze(2).broadcast_to([128, 7, 64]), Alu.mult)
                nc.vector.tensor_tensor(
                    attn_pair[:, 7:9, r0:r0 + 64], accB,
                    recB[:].unsqueeze(2).broadcast_to([128, 2, 64]), Alu.mult)
                if ph % 2 == 1:
                    pr = ph // 2
                    if pr < 3:
                        for qt in range(NQT):
                            nc.sync.dma_start_transpose(
                                attnT[:, pr, qt * 128:(qt + 1) * 128],
                                attn_pair[:, qt, :])
                    else:
                        # tail pair: PE-mode transpose (PE is idle here; the
                        # DMA-transpose path would serialize ~11us at the tail)
                        tps = psB.tile([128, NQT, 128], bf16, name="tps",
                                       tag="avacc")
                        for qt in range(NQT):
                            nc.tensor.transpose(
                                tps[:, qt, :], attn_pair[:, qt, :], ident_sb[:])
                        nc.vector.tensor_copy(
                            attnT[:, pr, :],
                            tps[:].rearrange("p a b -> p (a b)"))

            avail3 = {"kts": 0}  # exp coverage (in ktiles) of pair 3's buffers

            def av_pair_gen(E, ebuf, obuf, avail=None):
                # full AV + normalize for a head pair, yielded in ~2-ktile
                # chunks so the op loop can spread the matmuls evenly.
                # Both heads share the single 2-bank accumulator sequentially.
                for (h, bf) in ((E, ebuf), (E + 1, obuf)):
                    acc = psB.tile([128, 642], f32, name="avacc", tag="avacc")
                    # matmul start=True clears the whole bank's has_written
                    # bits, which would wipe sibling accumulation groups in
                    # the same bank -- zero and accumulate with start=False.
                    nc.vector.memset(acc[:], 0.0)
                    t = 0
                    while t < NKT:
                        if avail is not None and t >= avail["kts"]:
                            yield  # blocked on exp coverage; no work emitted
                            continue
                        for qt in range(NQT):
                            c0 = _accum_col(qt)
                            nc.tensor.matmul(
                                acc[:, c0:c0 + 65],
                                expt[:, bf,
                                     t * LQ + qt * 128: t * LQ + (qt + 1) * 128],
                                v_sb[:, t, h * 65:(h + 1) * 65],
                                start=False, stop=(t == NKT - 1),
                                skip_group_check=True)
                        t += 1
                        if t % 2 == 0:
                            yield
                    norm_head(h, acc)
                    yield

            pumps = []

            def pump_step():
                while pumps:
                    if next(pumps[0], "done") == "done":
                        pumps.pop(0)
                        continue
                    return

            for pr in range(4):
                E = 2 * pr
                ebuf, obuf = (pr % 2) * 2, (pr % 2) * 2 + 1
                if pr == 3:
                    # queued behind pair 2's generator; starts mid-window once
                    # that drains, gated by this window's exp coverage
                    pumps.append(av_pair_gen(E, ebuf, obuf, avail=avail3))
                for s in range(N_OPS):
                    g = pr * N_OPS + s
                    while units and units[0][0] <= g:
                        units.pop(0)[1]()
                    base = s * EXP_OP
                    end = min(base + EXP_OP, STREAM)
                    stagE = psA.tile([128, EXP_OP], f32, name="stagE", tag="stag")
                    stagO = psA.tile([128, EXP_OP], f32, name="stagO", tag="stag")
                    pts = sorted({base, end}
                                 | set(range(base + 512, end, 512))
                                 | {k * LQ for k in range(1, NKT)
                                    if base < k * LQ < end})
                    segs = list(zip(pts, pts[1:]))
                    segs.sort(key=lambda ab: (ab[0] // LQ, ab[0]))
                    for (a, b) in segs:
                        t, q0 = a // LQ, a % LQ
                        for (r0, stg) in ((0, stagE), (64, stagO)):
                            nc.tensor.matmul(
                                stg[:, a - base:b - base],
                                k_sb[r0:r0 + 64, pr, t * 128:(t + 1) * 128],
                                q_sb[r0:r0 + 64, pr, q0:q0 + (b - a)],
                                start=True, stop=True, tile_position=(r0, 0))
                    nc.scalar.activation(
                        expt[:, ebuf, base:end], stagE[:, 0:end - base],
                        Act.Exp, scale=1.0 / np.sqrt(DK), bias=ebias_sb[:])
                    nc.scalar.activation(
                        expt[:, obuf, base:end], stagO[:, 0:end - base],
                        Act.Exp, scale=1.0 / np.sqrt(DK), bias=ebias_sb[:])
                    if pr == 3:
                        # ops 0..s-1 are fully consumed by ACT one op later;
                        # safe stall-free coverage for same-window AV
                        avail3["kts"] = (s * EXP_OP) // LQ
                    # pump AV work: depends only on already-finished exp
                    # output, so the PE never waits on in-flight ACTIVATEs
                    for _ in range(3):
                        pump_step()
                if pr < 3:
                    while pumps:
                        pump_step()
                    pumps.append(av_pair_gen(E, ebuf, obuf))

            # ======= tail: drain last pair interleaved with the projection ===
            avail3["kts"] = NKT
            ps_y = []
            proj_chunks = []
            for ct in range(2):
                ps = psA.tile([128, LQ], f32, name="yps", tag="stag")
                ps_y.append(ps)
                for dtl in range(3):
                    for (q0, qn) in ((0, 512), (512, 512), (1024, 128)):
                        proj_chunks.append((ct, dtl, q0, qn))

            def proj_mm(ct, dtl, q0, qn):
                nc.tensor.matmul(
                    ps_y[ct][:, q0:q0 + qn],
                    wot_sb[:, dtl, ct * 128:(ct + 1) * 128],
                    attnT[:, dtl, q0:q0 + qn],
                    start=(dtl == 0), stop=False)

            while pumps or proj_chunks:
                pump_step()
                if not pumps:
                    while proj_chunks:
                        proj_mm(*proj_chunks.pop(0))
                for _ in range(3):
                    if proj_chunks:
                        proj_mm(*proj_chunks.pop(0))

            while units:
                units.pop(0)[1]()

            # dtl3 (waits on the last pair's attnT) + stats per ct
            sq_scr = scr.tile([128, LQ], f32, name="sq", tag="sq")
            for ct in range(2):
                for (q0, qn) in ((0, 512), (512, 512), (1024, 128)):
                    nc.tensor.matmul(
                        ps_y[ct][:, q0:q0 + qn],
                        wot_sb[:, 3, ct * 128:(ct + 1) * 128],
                        attnT[:, 3, q0:q0 + qn],
                        start=False, stop=True)
                nc.vector.tensor_reduce(
                    stats[:, 2 * ct:2 * ct + 1], ps_y[ct][:],
                    mybir.AxisListType.X, Alu.add)
                # sumsq on the (now idle) scalar engine: Square + accumulate
                nc.scalar.activation(
                    sq_scr[:], ps_y[ct][:], Act.Square,
                    accum_out=stats[:, 2 * ct + 1:2 * ct + 2])

            # ================= AllReduce of stats =================
            cin = dram.tile([128, 4], f32)
            cout = dram.tile([128, 4], f32, addr_space="Shared")
            nc.sync.dma_start(cin[:], stats[:])
            nc.gpsimd.collective_compute(
                "AllReduce", Alu.add,
                replica_groups=[list(range(N_CORES))],
                ins=[cin.opt()], outs=[cout.opt()])
            nc.sync.dma_start(gstats[:], cout[:])

            # ================= BN coefficients =================
            # rstd = exp(-0.5*ln(var+eps)): Ln+Exp share one ACT table set, so
            # no mid-kernel table reload (Sqrt would force one)
            mean = scr.tile([128, 2], f32, name="mean")
            m2 = scr.tile([128, 2], f32, name="m2")
            var = scr.tile([128, 2], f32, name="var")
            lnv = scr.tile([128, 2], f32, name="lnv")
            rstd = scr.tile([128, 2], f32, name="rstd")
            Ac = scr.tile([128, 2], f32, name="Ac")
            Bc = scr.tile([128, 2], f32, name="Bc")
            gs = gstats[:].rearrange("p (c two) -> p c two", two=2)
            nc.vector.tensor_scalar(mean[:], gs[:, :, 0], 1.0 / NSAMP, None, Alu.mult)
            nc.vector.tensor_scalar(m2[:], gs[:, :, 1], 1.0 / NSAMP, None, Alu.mult)
            nc.vector.tensor_tensor(var[:], mean[:], mean[:], Alu.mult)
            nc.vector.tensor_tensor(var[:], m2[:], var[:], Alu.subtract)
            nc.vector.tensor_scalar(var[:], var[:], BN_EPS, None, Alu.add)
            nc.scalar.activation(lnv[:], var[:], Act.Ln)
            nc.scalar.activation(rstd[:], lnv[:], Act.Exp, scale=-0.5)
            nc.vector.tensor_tensor(Ac[:], bnw_sb[:], rstd[:], Alu.mult)
            nc.vector.tensor_tensor(Bc[:], mean[:], Ac[:], Alu.mult)
            nc.vector.tensor_tensor(Bc[:], bnb_sb[:], Bc[:], Alu.subtract)

            # ========== apply + residual + leaky relu (ACT/DVE/Pool split) ====
            # affine on the scalar engine (idle now), residual+leaky split by
            # columns between vector (fast) and gpsimd (slower) engines
            HV = LQ // 2  # two column chunks so the output DMA overlaps the apply
            for ct in range(2):
                z = scr.tile([128, LQ], f32, name="z", tag=f"z{ct}")
                r = scr.tile([128, LQ], f32, name="r", tag=f"r{ct}")
                nc.scalar.activation(z[:], ps_y[ct][:], Act.Identity,
                                     bias=Bc[:, ct:ct + 1], scale=Ac[:, ct:ct + 1])
                for (c0, c1) in ((0, HV), (HV, LQ)):
                    nc.vector.tensor_tensor(r[:, c0:c1], z[:, c0:c1],
                                            xq_sb[:, ct, c0:c1], Alu.add)
                    nc.vector.scalar_tensor_tensor(
                        z[:, c0:c1], r[:, c0:c1], NEG, r[:, c0:c1],
                        Alu.mult, Alu.max)
                    nc.sync.dma_start(y_d[ct * 128:(ct + 1) * 128, c0:c1],
                                      z[:, c0:c1])

    nc.compile()
    return nc


def _prep_inputs(x, Wq, Wk, Wv, Wo, bn_w, bn_b, gamma):
    x = np.asarray(x, np.float32)
    Wq = np.asarray(Wq, np.float32)
    Wk = np.asarray(Wk, np.float32)
    Wv = np.asarray(Wv, np.float32)
    Wo = np.asarray(Wo, np.float32)
    bn_w = np.asarray(bn_w, np.float32)
    bn_b = np.asarray(bn_b, np.float32)
    gamma = np.asarray(gamma, np.float32)

    xf = x.reshape(B, C, L)
    bf = ml_dtypes.bfloat16
    wqt = np.ascontiguousarray(Wq.T).astype(bf)
    wkt = np.ascontiguousarray(Wk.T).astype(bf)
    wvt = np.zeros((C, NH * 65), np.float32)
    wvtT = Wv.T  # [C, D]
    for h in range(NH):
        wvt[:, h * 65:h * 65 + 64] = wvtT[:, h * 64:(h + 1) * 64]
    wvt = wvt.astype(bf)
    wot = np.ascontiguousarray(Wo.T).astype(bf)
    g = float(gamma[0])
    bnw2 = np.ascontiguousarray((g * bn_w).reshape(2, 128).T)
    bnb2 = np.ascontiguousarray((g * bn_b).reshape(2, 128).T)
    ident = np.eye(128, dtype=bf)

    in_maps = []
    for c in range(N_CORES):
        b, s = c // 2, c % 2
        xb = np.ascontiguousarray(xf[b]).astype(bf)
        in_maps.append({
            "xf": xb,
            "xq": np.ascontiguousarray(xb[:, s * LQ:(s + 1) * LQ]),
            "wqt": wqt, "wkt": wkt, "wvt": wvt, "wot": wot,
            "bnw2": bnw2, "bnb2": bnb2, "ident": ident,
        })
    return in_maps


def kernel(x, Wq, Wk, Wv, Wo, bn_w, bn_b, gamma):
    # NTFF profile hook (needed only when TRACE=True, harmless otherwise)
    if "antenv.axon_hooks" not in sys.modules:
        try:
            import trn_agent_boot.trn_boot as _tb
            _h = _tb._ntff_profile_via_ctypes("/opt/axon/libaxon_pjrt.so")
            _m = types.ModuleType("antenv.axon_hooks")
            _m.get_axon_ntff_profile_hook = lambda: _h
            _m.set_axon_ntff_profile_hook = lambda hh: None
            sys.modules["antenv.axon_hooks"] = _m
        except Exception:
            pass

    from concourse import bass_utils

    if "nc" not in _cache:
        _cache["nc"] = _build()
    nc = _cache["nc"]

    in_maps = _prep_inputs(x, Wq, Wk, Wv, Wo, bn_w, bn_b, gamma)
    res = bass_utils.run_bass_kernel_spmd(
        nc, in_maps, core_ids=list(range(N_CORES)), trace=TRACE)
    _cache["last_result"] = res

    out = np.empty((B, C, L), np.float32)
    for c in range(N_CORES):
        b, s = c // 2, c % 2
        out[b][:, s * LQ:(s + 1) * LQ] = res.results[c]["y"]
    return out.reshape(B, C, HH, WW)


# revision 28
# speedup vs baseline: 1.0378x; 1.0378x over previous
"""Trainium2 Bass kernel for MultiHeadSelfAttention + BN + residual + LeakyReLU.

Sharding: 8 cores = (batch b, token-half s); each core computes all 8 heads of
attention for its 1152 query tokens of its batch, the full output projection
for those tokens, and BN via a single all-core AllReduce of per-channel
sum/sumsq statistics.

v1 design: the scalar engine's exp stream (166k columns ~ 140us) is the hard
floor; everything else (matmuls, copies, transposes, DMA) is paced to hide
under it. fp8e4 exp output makes the AV weight loads cheap enough that the
tensor engine stays below the scalar engine's rate.
"""

import sys
import types

if "/opt/trn_rl_repo" not in sys.path:
    sys.path.insert(0, "/opt/trn_rl_repo")

import numpy as np
import ml_dtypes

N_CORES = 8
B, C, HH, WW = 4, 256, 48, 48
L = HH * WW            # 2304 tokens per batch
NH, DK = 8, 64
D = NH * DK            # 512
LQ = L // 2            # 1152 query tokens per core
NKT = L // 128         # 18 key tiles
NQT = LQ // 128        # 9 query tiles
STREAM = NKT * LQ      # 20736 logit columns per head
EXP_OP = 1536          # columns per exp ACTIVATE (3 PSUM banks)
N_OPS = (STREAM + EXP_OP - 1) // EXP_OP  # 14 (last op = 768 cols)
NSAMP = B * L          # 9216 samples per channel for BN
BN_EPS = 1e-5
NEG = 0.01

TRACE = False
EXPT_F8 = True         # exp stream dtype: fp8e4 (False -> bf16)

_cache = {}


def _accum_col(qt):
    # packed AV accumulator columns; avoid crossing the 512-col PSUM bank edge
    return qt * 65 if qt < 7 else 512 + (qt - 7) * 65


def _build():
    import concourse.bacc as bacc
    import concourse.mybir as mybir
    import concourse.tile as tile

    dt = mybir.dt
    f32, bf16 = dt.float32, dt.bfloat16
    f8 = dt.float8e4
    e_dt = f8 if EXPT_F8 else bf16
    # exp(x-2) instead of exp(x): scales numerator AND denominator by e^-2
    # (cancels in the softmax) and keeps the max weight ~e^3 well under the
    # fp8e4 max, avoiding overflow->inf->NaN
    EXP_BIAS = -2.0 if EXPT_F8 else 0.0
    Alu = mybir.AluOpType
    Act = mybir.ActivationFunctionType

    nc = bacc.Bacc(trn_type="TRN2", num_devices=N_CORES, debug=False)

    # ---- DRAM I/O ----
    xf_d = nc.dram_tensor("xf", [C, L], bf16, kind="ExternalInput").ap()
    xq_d = nc.dram_tensor("xq", [C, LQ], bf16, kind="ExternalInput").ap()
    wqt_d = nc.dram_tensor("wqt", [C, D], bf16, kind="ExternalInput").ap()
    wkt_d = nc.dram_tensor("wkt", [C, D], bf16, kind="ExternalInput").ap()
    wvt_d = nc.dram_tensor("wvt", [C, NH * 65], bf16, kind="ExternalInput").ap()
    wot_d = nc.dram_tensor("wot", [D, C], bf16, kind="ExternalInput").ap()
    bnw_d = nc.dram_tensor("bnw2", [128, 2], f32, kind="ExternalInput").ap()
    bnb_d = nc.dram_tensor("bnb2", [128, 2], f32, kind="ExternalInput").ap()
    id_d = nc.dram_tensor("ident", [128, 128], bf16, kind="ExternalInput").ap()
    y_d = nc.dram_tensor("y", [C, LQ], f32, kind="ExternalOutput").ap()

    with tile.TileContext(nc) as tc:
        with (
            tc.tile_pool(name="const", bufs=1) as cpool,
            tc.tile_pool(name="big", bufs=1) as big,
            tc.tile_pool(name="pair", bufs=2) as pairp,
            tc.tile_pool(name="scr", bufs=2) as scr,
            tc.tile_pool(name="psA", bufs=2, space="PSUM") as psA,
            tc.tile_pool(name="psB", bufs=1, space="PSUM") as psB,
            tc.tile_pool(name="dram", bufs=1, space="DRAM") as dram,
        ):
            # ---- SBUF constants / inputs ----
            xf_sb = cpool.tile([128, 2, L], bf16)
            xq_sb = cpool.tile([128, 2, LQ], bf16)
            wqt_sb = cpool.tile([128, 2, D], bf16)
            wkt_sb = cpool.tile([128, 2, D], bf16)
            wvt_sb = cpool.tile([128, 2, NH * 65], bf16)
            wot_sb = cpool.tile([128, 4, C], bf16)
            bnw_sb = cpool.tile([128, 2], f32)
            bnb_sb = cpool.tile([128, 2], f32)
            ident_sb = cpool.tile([128, 128], bf16)
            ebias_sb = cpool.tile([128, 1], f32)
            nc.vector.memset(ebias_sb[:], EXP_BIAS)

            # weights + query tokens first (first QKV units need them)
            for ct in range(2):
                nc.sync.dma_start(wkt_sb[:, ct, :], wkt_d[ct * 128:(ct + 1) * 128, :])
                nc.sync.dma_start(wqt_sb[:, ct, :], wqt_d[ct * 128:(ct + 1) * 128, :])
                nc.sync.dma_start(wvt_sb[:, ct, :], wvt_d[ct * 128:(ct + 1) * 128, :])
                nc.sync.dma_start(xq_sb[:, ct, :], xq_d[ct * 128:(ct + 1) * 128, :])
                # xf in quarters for earlier availability of early key tiles
                for qchunk in range(4):
                    c0 = qchunk * (L // 4)
                    nc.sync.dma_start(xf_sb[:, ct, c0:c0 + L // 4],
                                      xf_d[ct * 128:(ct + 1) * 128, c0:c0 + L // 4])
            for dtl in range(4):
                nc.sync.dma_start(wot_sb[:, dtl, :], wot_d[dtl * 128:(dtl + 1) * 128, :])
            nc.sync.dma_start(bnw_sb[:], bnw_d[:])
            nc.sync.dma_start(bnb_sb[:], bnb_d[:])
            nc.sync.dma_start(ident_sb[:], id_d[:])

            # warm-up AllReduce: absorbs the collective engine's ~11us
            # startup latency (and roughly aligns the cores) while the body
            # runs, so the real stats AllReduce at the tail starts fast
            warm_in = dram.tile([128, 1], f32)
            warm_out = dram.tile([128, 1], f32, addr_space="Shared")
            nc.sync.dma_start(warm_in[:], ebias_sb[:])
            nc.gpsimd.collective_compute(
                "AllReduce", Alu.add,
                replica_groups=[list(range(N_CORES))],
                ins=[warm_in.opt()], outs=[warm_out.opt()])

            # ---- big SBUF tensors ----
            k_sb = big.tile([128, 4, L], bf16)        # K: [dpair, pair, kpos]
            q_sb = big.tile([128, 4, LQ], bf16)       # Q
            v_sb = big.tile([128, NKT, NH * 65], bf16)  # V token-major + ones cols
            # 4 exp-stream buffers: pair p writes bufs (p%2)*2+{0,1}; its AV
            # (run one pair-window later) still reads them while the next pair
            # fills the other two
            expt = big.tile([128, 4, STREAM], e_dt)
            attnT = big.tile([128, 4, LQ], bf16)      # transposed attention output
            stats = big.tile([128, 4], f32)
            gstats = big.tile([128, 4], f32)

            # ================= QKV background units =================
            def emit_k_half(dtl, half):
                ps = psA.tile([128, LQ], f32, name="qkvps", tag="stag")
                for ct in range(2):
                    for (q0, qn) in ((0, 512), (512, 512), (1024, 128)):
                        nc.tensor.matmul(
                            ps[:, q0:q0 + qn],
                            wkt_sb[:, ct, dtl * 128:(dtl + 1) * 128],
                            xf_sb[:, ct, half * LQ + q0: half * LQ + q0 + qn],
                            start=(ct == 0), stop=(ct == 1))
                nc.vector.tensor_copy(k_sb[:, dtl, half * LQ:(half + 1) * LQ], ps[:])

            def emit_q_tile(dtl):
                ps = psA.tile([128, LQ], f32, name="qkvps", tag="stag")
                for ct in range(2):
                    for (q0, qn) in ((0, 512), (512, 512), (1024, 128)):
                        nc.tensor.matmul(
                            ps[:, q0:q0 + qn],
                            wqt_sb[:, ct, dtl * 128:(dtl + 1) * 128],
                            xq_sb[:, ct, q0:q0 + qn],
                            start=(ct == 0), stop=(ct == 1))
                nc.vector.tensor_copy(q_sb[:, dtl, :], ps[:])

            def emit_v_ltile(lt):
                ps = psA.tile([128, LQ], f32, name="qkvps", tag="stag")
                for ct in range(2):
                    for (q0, qn) in ((0, 512), (512, 8)):
                        nc.tensor.matmul(
                            ps[:, q0:q0 + qn],
                            xf_sb[:, ct, lt * 128:(lt + 1) * 128],
                            wvt_sb[:, ct, q0:q0 + qn],
                            start=(ct == 0), stop=(ct == 1))
                nc.vector.tensor_copy(v_sb[:, lt, :], ps[:, 0:520])
                # ones columns for the softmax denominator
                nc.vector.memset(v_sb[:, lt, 64::65], 1.0)

            # deadline-ordered background units (global op index g = pr*N_OPS+s;
            # a unit is emitted before the logits of op g). AV of pair pr runs
            # during window pr+1, so V tiles are only consumed from window 1 on.
            units = []
            for lt in range(1, NKT):
                units.append((1 + (2 * lt) // 3, lambda lt=lt: emit_v_ltile(lt)))
            units.append((6, lambda: emit_k_half(0, 1)))
            for pr in range(1, 4):
                units.append((pr * N_OPS - 10, lambda p=pr: emit_k_half(p, 0)))
                units.append((pr * N_OPS - 7, lambda p=pr: emit_q_tile(p)))
                units.append((pr * N_OPS + 6, lambda p=pr: emit_k_half(p, 1)))
            units.sort(key=lambda t: t[0])

            # prologue: what head 0 op 0 needs immediately
            emit_k_half(0, 0)
            emit_q_tile(0)
            emit_v_ltile(0)

            # ================= attention =================
            # AV consumption lags the exp stream by one ACT op, and each
            # head's final ktile + normalize happen inside the NEXT head's
            # window -- the PE never waits on the in-flight ACTIVATE, stays
            # gapless, and the HAM clock gate can hold 2.4GHz.
            apair = {"tile": None}

            def norm_head(ph, acc):
                r0 = (ph % 2) * 64
                if ph % 2 == 0:
                    apair["tile"] = pairp.tile([128, NQT, 128], bf16,
                                               name="apair", tag="apair")
                attn_pair = apair["tile"]
                recA = scr.tile([128, 7], f32, name="recA", tag="recA")
                recB = scr.tile([128, 2], f32, name="recB", tag="recB")
                nc.vector.reciprocal(recA[:], acc[:, 64:64 + 7 * 65:65])
                nc.vector.reciprocal(recB[:], acc[:, 512 + 64:512 + 2 * 65:65])
                accA = acc[:, 0:7 * 65].rearrange("p (q d) -> p q d", d=65)[:, :, 0:64]
                accB = acc[:, 512:512 + 2 * 65].rearrange("p (q d) -> p q d", d=65)[:, :, 0:64]
                nc.vector.tensor_tensor(
                    attn_pair[:, 0:7, r0:r0 + 64], accA,
                    recA[:].unsqueeze(2).broadcast_to([128, 7, 64]), Alu.mult)
                nc.vector.tensor_tensor(
                    attn_pair[:, 7:9, r0:r0 + 64], accB,
                    recB[:].unsqueeze(2).broadcast_to([128, 2, 64]), Alu.mult)
                if ph % 2 == 1:
                    pr = ph // 2
                    if pr < 3:
                        for qt in range(NQT):
                            nc.sync.dma_start_transpose(
                                attnT[:, pr, qt * 128:(qt + 1) * 128],
                                attn_pair[:, qt, :])
                    else:
                        # tail pair: PE-mode transpose (PE is idle here; the
                        # DMA-transpose path would serialize ~11us at the tail)
                        tps = psB.tile([128, NQT, 128], bf16, name="tps",
                                       tag="avacc")
                        for qt in range(NQT):
                            nc.tensor.transpose(
                                tps[:, qt, :], attn_pair[:, qt, :], ident_sb[:])
                        nc.vector.tensor_copy(
                            attnT[:, pr, :],
                            tps[:].rearrange("p a b -> p (a b)"))

            avail3 = {"kts": 0}  # exp coverage (in ktiles) of pair 3's buffers

            def av_pair_gen(E, ebuf, obuf, avail=None):
                # full AV + normalize for a head pair, yielded in ~2-ktile
                # chunks so the op loop can spread the matmuls evenly.
                # Both heads share the single 2-bank accumulator sequentially.
                for (h, bf) in ((E, ebuf), (E + 1, obuf)):
                    acc = psB.tile([128, 642], f32, name="avacc", tag="avacc")
                    # matmul start=True clears the whole bank's has_written
                    # bits, which would wipe sibling accumulation groups in
                    # the same bank -- zero and accumulate with start=False.
                    nc.vector.memset(acc[:], 0.0)
                    t = 0
                    while t < NKT:
                        if avail is not None and t >= avail["kts"]:
                            yield  # blocked on exp coverage; no work emitted
                            continue
                        for qt in range(NQT):
                            c0 = _accum_col(qt)
                            nc.tensor.matmul(
                                acc[:, c0:c0 + 65],
                                expt[:, bf,
                                     t * LQ + qt * 128: t * LQ + (qt + 1) * 128],
                                v_sb[:, t, h * 65:(h + 1) * 65],
                                start=False, stop=(t == NKT - 1),
                                skip_group_check=True)
                        t += 1
                        if t % 2 == 0:
                            yield
                    norm_head(h, acc)
                    yield

            pumps = []

            def pump_step():
                while pumps:
                    if next(pumps[0], "done") == "done":
                        pumps.pop(0)
                        continue
                    return

            for pr in range(4):
                E = 2 * pr
                ebuf, obuf = (pr % 2) * 2, (pr % 2) * 2 + 1
                if pr == 3:
                    # queued behind pair 2's generator; starts mid-window once
                    # that drains, gated by this window's exp coverage
                    pumps.append(av_pair_gen(E, ebuf, obuf, avail=avail3))
                for s in range(N_OPS):
                    g = pr * N_OPS + s
                    while units and units[0][0] <= g:
                        units.pop(0)[1]()
                    base = s * EXP_OP
                    end = min(base + EXP_OP, STREAM)
                    stagE = psA.tile([128, EXP_OP], f32, name="stagE", tag="stag")
                    stagO = psA.tile([128, EXP_OP], f32, name="stagO", tag="stag")
                    pts = sorted({base, end}
                                 | set(range(base + 512, end, 512))
                                 | {k * LQ for k in range(1, NKT)
                                    if base < k * LQ < end})
                    segs = list(zip(pts, pts[1:]))
                    segs.sort(key=lambda ab: (ab[0] // LQ, ab[0]))
                    for (a, b) in segs:
                        t, q0 = a // LQ, a % LQ
                        for (r0, stg) in ((0, stagE), (64, stagO)):
                            nc.tensor.matmul(
                                stg[:, a - base:b - base],
                                k_sb[r0:r0 + 64, pr, t * 128:(t + 1) * 128],
                                q_sb[r0:r0 + 64, pr, q0:q0 + (b - a)],
                                start=True, stop=True, tile_position=(r0, 0))
                    nc.scalar.activation(
                        expt[:, ebuf, base:end], stagE[:, 0:end - base],
                        Act.Exp, scale=1.0 / np.sqrt(DK), bias=ebias_sb[:])
                    nc.scalar.activation(
                        expt[:, obuf, base:end], stagO[:, 0:end - base],
                        Act.Exp, scale=1.0 / np.sqrt(DK), bias=ebias_sb[:])
                    if pr == 3:
                        # ops 0..s-1 are fully consumed by ACT one op later;
                        # safe stall-free coverage for same-window AV
                        avail3["kts"] = (s * EXP_OP) // LQ
                    # pump AV work: depends only on already-finished exp
                    # output, so the PE never waits on in-flight ACTIVATEs
                    for _ in range(3):
                        pump_step()
                if pr < 3:
                    while pumps:
                        pump_step()
                    pumps.append(av_pair_gen(E, ebuf, obuf))

            # ======= tail: drain last pair interleaved with the projection ===
            avail3["kts"] = NKT
            ps_y = []
            proj_chunks = []
            for ct in range(2):
                ps = psA.tile([128, LQ], f32, name="yps", tag="stag")
                ps_y.append(ps)
                for dtl in range(3):
                    for (q0, qn) in ((0, 512), (512, 512), (1024, 128)):
                        proj_chunks.append((ct, dtl, q0, qn))

            def proj_mm(ct, dtl, q0, qn):
                nc.tensor.matmul(
                    ps_y[ct][:, q0:q0 + qn],
                    wot_sb[:, dtl, ct * 128:(ct + 1) * 128],
                    attnT[:, dtl, q0:q0 + qn],
                    start=(dtl == 0), stop=False)

            while pumps or proj_chunks:
                pump_step()
                if not pumps:
                    while proj_chunks:
                        proj_mm(*proj_chunks.pop(0))
                for _ in range(3):
                    if proj_chunks:
                        proj_mm(*proj_chunks.pop(0))

            while units:
                units.pop(0)[1]()

            # dtl3 (waits on the last pair's attnT) + stats per ct
            sq_scr = scr.tile([128, LQ], f32, name="sq", tag="sq")
            for ct in range(2):
                for (q0, qn) in ((0, 512), (512, 512), (1024, 128)):
                    nc.tensor.matmul(
                        ps_y[ct][:, q0:q0 + qn],
                        wot_sb[:, 3, ct * 128:(ct + 1) * 128],
                        attnT[:, 3, q0:q0 + qn],
                        start=False, stop=True)
                nc.vector.tensor_reduce(
                    stats[:, 2 * ct:2 * ct + 1], ps_y[ct][:],
                    mybir.AxisListType.X, Alu.add)
                # sumsq on the (now idle) scalar engine: Square + accumulate
                nc.scalar.activation(
                    sq_scr[:], ps_y[ct][:], Act.Square,
                    accum_out=stats[:, 2 * ct + 1:2 * ct + 2])

            # ================= AllReduce of stats =================
            cin = dram.tile([128, 4], f32)
            cout = dram.tile([128, 4], f32, addr_space="Shared")
            nc.sync.dma_start(cin[:], stats[:])
            nc.gpsimd.collective_compute(
                "AllReduce", Alu.add,
                replica_groups=[list(range(N_CORES))],
                ins=[cin.opt()], outs=[cout.opt()])
            nc.sync.dma_start(gstats[:], cout[:])

            # ================= BN coefficients =================
            # rstd = exp(-0.5*ln(var+eps)): Ln+Exp share one ACT table set, so
            # no mid-kernel table reload (Sqrt would force one)
            mean = scr.tile([128, 2], f32, name="mean")
            m2 = scr.tile([128, 2], f32, name="m2")
            var = scr.tile([128, 2], f32, name="var")
            lnv = scr.tile([128, 2], f32, name="lnv")
            rstd = scr.tile([128, 2], f32, name="rstd")
            Ac = scr.tile([128, 2], f32, name="Ac")
            Bc = scr.tile([128, 2], f32, name="Bc")
            gs = gstats[:].rearrange("p (c two) -> p c two", two=2)
            nc.vector.tensor_scalar(mean[:], gs[:, :, 0], 1.0 / NSAMP, None, Alu.mult)
            nc.vector.tensor_scalar(m2[:], gs[:, :, 1], 1.0 / NSAMP, None, Alu.mult)
            nc.vector.tensor_tensor(var[:], mean[:], mean[:], Alu.mult)
            nc.vector.tensor_tensor(var[:], m2[:], var[:], Alu.subtract)
            nc.vector.tensor_scalar(var[:], var[:], BN_EPS, None, Alu.add)
            nc.scalar.activation(lnv[:], var[:], Act.Ln)
            nc.scalar.activation(rstd[:], lnv[:], Act.Exp, scale=-0.5)
            nc.vector.tensor_tensor(Ac[:], bnw_sb[:], rstd[:], Alu.mult)
            nc.vector.tensor_tensor(Bc[:], mean[:], Ac[:], Alu.mult)
            nc.vector.tensor_tensor(Bc[:], bnb_sb[:], Bc[:], Alu.subtract)

            # ========== apply + residual + leaky relu (ACT/DVE/Pool split) ====
            # affine on the scalar engine (idle now), residual+leaky split by
            # columns between vector (fast) and gpsimd (slower) engines
            HV = LQ // 2  # two column chunks so the output DMA overlaps the apply
            for ct in range(2):
                z = scr.tile([128, LQ], f32, name="z", tag=f"z{ct}")
                r = scr.tile([128, LQ], f32, name="r", tag=f"r{ct}")
                nc.scalar.activation(z[:], ps_y[ct][:], Act.Identity,
                                     bias=Bc[:, ct:ct + 1], scale=Ac[:, ct:ct + 1])
                for (c0, c1) in ((0, HV), (HV, LQ)):
                    nc.vector.tensor_tensor(r[:, c0:c1], z[:, c0:c1],
                                            xq_sb[:, ct, c0:c1], Alu.add)
                    nc.vector.scalar_tensor_tensor(
                        z[:, c0:c1], r[:, c0:c1], NEG, r[:, c0:c1],
                        Alu.mult, Alu.max)
                    nc.sync.dma_start(y_d[ct * 128:(ct + 1) * 128, c0:c1],
                                      z[:, c0:c1])

    nc.compile()
    return nc


def _prep_inputs(x, Wq, Wk, Wv, Wo, bn_w, bn_b, gamma):
    x = np.asarray(x, np.float32)
    Wq = np.asarray(Wq, np.float32)
    Wk = np.asarray(Wk, np.float32)
    Wv = np.asarray(Wv, np.float32)
    Wo = np.asarray(Wo, np.float32)
    bn_w = np.asarray(bn_w, np.float32)
    bn_b = np.asarray(bn_b, np.float32)
    gamma = np.asarray(gamma, np.float32)

    xf = x.reshape(B, C, L)
    bf = ml_dtypes.bfloat16
    wqt = np.ascontiguousarray(Wq.T).astype(bf)
    wkt = np.ascontiguousarray(Wk.T).astype(bf)
    wvt = np.zeros((C, NH * 65), np.float32)
    wvtT = Wv.T  # [C, D]
    for h in range(NH):
        wvt[:, h * 65:h * 65 + 64] = wvtT[:, h * 64:(h + 1) * 64]
    wvt = wvt.astype(bf)
    wot = np.ascontiguousarray(Wo.T).astype(bf)
    g = float(gamma[0])
    bnw2 = np.ascontiguousarray((g * bn_w).reshape(2, 128).T)
    bnb2 = np.ascontiguousarray((g * bn_b).reshape(2, 128).T)
    ident = np.eye(128, dtype=bf)

    in_maps = []
    for c in range(N_CORES):
        b, s = c // 2, c % 2
        xb = np.ascontiguousarray(xf[b]).astype(bf)
        in_maps.append({
            "xf": xb,
            "xq": np.ascontiguousarray(xb[:, s * LQ:(s + 1) * LQ]),
            "wqt": wqt, "wkt": wkt, "wvt": wvt, "wot": wot,
            "bnw2": bnw2, "bnb2": bnb2, "ident": ident,
        })
    return in_maps


def kernel(x, Wq, Wk, Wv, Wo, bn_w, bn_b, gamma):
    # NTFF profile hook (needed only when TRACE=True, harmless otherwise)
    if "antenv.axon_hooks" not in sys.modules:
        try:
            import trn_agent_boot.trn_boot as _tb
            _h = _tb._ntff_profile_via_ctypes("/opt/axon/libaxon_pjrt.so")
            _m = types.ModuleType("antenv.axon_hooks")
            _m.get_axon_ntff_profile_hook = lambda: _h
            _m.set_axon_ntff_profile_hook = lambda hh: None
            sys.modules["antenv.axon_hooks"] = _m
        except Exception:
            pass

    from concourse import bass_utils

    if "nc" not in _cache:
        _cache["nc"] = _build()
    nc = _cache["nc"]

    in_maps = _prep_inputs(x, Wq, Wk, Wv, Wo, bn_w, bn_b, gamma)
    res = bass_utils.run_bass_kernel_spmd(
        nc, in_maps, core_ids=list(range(N_CORES)), trace=TRACE)
    _cache["last_result"] = res

    out = np.empty((B, C, L), np.float32)
    for c in range(N_CORES):
        b, s = c // 2, c % 2
        out[b][:, s * LQ:(s + 1) * LQ] = res.results[c]["y"]
    return out.reshape(B, C, HH, WW)
